# revision 1
# baseline (speedup 1.0000x reference)
"""BinaryDense kernel for Trainium2: out = sign(x) @ sign(w).

Full shapes: x [8192, 4096] f32, w [4096, 4096] f32 -> out [8192, 4096] f32.
Sharding over 8 NeuronCores: x rows split 4 ways, w columns split 2 ways;
each core computes a [2048, 2048] block.  No collectives.

Core ideas (cost-model-driven):
  - fp8e5 cast-loads (SWDGE): IEEE casts preserve the sign BIT (even on
    underflow to +-0) and only the sign bit matters -> input DMA halves.
  - Bitwise sign on uint16 views: (r & 0x8080) | 0x3C3C == +-1.0 fp8e5 in
    both packed bytes.  Single DVE op per 2 elements, exact.
  - Matmuls: fp8 DoubleRowSwInterleave (0.5 cycles/row).  The k-pair-packed
    transposed x IS the interleaved stationary operand; w cast-loads land
    directly in the plane-separated moving layout [p, j, t, n].  The mode
    reads stationary columns in reverse order, so the host pre-reverses x
    rows within each 128-row block.
  - PSUM f32 accumulation is exact (+-1 products); int16 out; host widens.

Schedule: the finish time is ~(w-stream end + 97us), so the w stream runs
UNBROKEN on the Pool/SWDGE queue right after the first x chunk.  The first
two x chunks are transposed on the PE (raw fp8 pairs as u16; the sign is
fused into the DVE psum->SBUF eviction), which costs PE time only where PE
is delivery-paced anyway and keeps cross-queue DMA hops out of the w
stream.  Remaining chunks use XBAR DMA-transposes after the stream, where
the DMA device has slack.  Steady-state matmuls run nb-major so psum banks
recycle incrementally (6 matmul banks + 2 transpose banks).

Queue map: Pool=cast loads | DVE=signs+evictions | SP=XBAR transposes |
ACT=output DMA issue | PE=matmuls + first-two-chunk transposes.
"""

import numpy as np

import concourse.mybir as mybir
import concourse.tile as tile
from concourse import bacc
from concourse.bass_utils import run_bass_kernel_spmd
from concourse.masks import make_identity

P = 128
N_CORES = 8
RM, RN = 4, 2
M_FULL, K, N_FULL = 8192, 4096, 4096
M_SH, N_SH = M_FULL // RM, N_FULL // RN   # 2048, 2048
MB = M_SH // P           # 16 m-blocks
JB = K // 256            # 16 k-groups (DoubleRow: 2 planes x 128)
NB = N_SH // 512         # 4 psum-width chunks
XC = MB // 2             # 8 x-chunks of 2 m-blocks
TGRP = 8                 # u16 128-blocks per PE-transpose psum group

F32 = mybir.dt.float32
FP8 = mybir.dt.float8e5
U16 = mybir.dt.uint16
I16 = mybir.dt.int16

AND_MASK = 0x8080
OR_MASK = 0x3C3C
DRSW = mybir.MatmulPerfMode.DoubleRowSwInterleave

_NC_CACHE = None

# DMA schedule pins in "ms" for tc.tile_wait_until (1e6 ns units)
PINS = {
    "xc1": 0.0264, "c2": 0.0294, "c3": 0.0324, "c4": 0.0354,
    "c5": 0.0423, "c6": 0.0488, "c7": 0.0517,
    "T2": 0.0387, "T3": 0.0452, "T4": 0.0546, "T5": 0.0614,
    "T6": 0.0674, "T7": 0.0720,
}


def build_nc():
    nc = bacc.Bacc("TRN2", target_bir_lowering=False, debug=False,
                   num_devices=N_CORES)
    x = nc.dram_tensor("x", [M_SH, K], F32, kind="ExternalInput").ap()
    w = nc.dram_tensor("w", [K, N_SH], F32, kind="ExternalInput").ap()
    out = nc.dram_tensor("out", [M_SH, N_SH], I16, kind="ExternalOutput").ap()

    with tile.TileContext(nc) as tc:
        with (
            tc.tile_pool(name="const", bufs=1) as const_pool,
            tc.tile_pool(name="xT", bufs=1) as xT_pool,
            tc.tile_pool(name="wbin", bufs=1) as w_pool,
            tc.tile_pool(name="xs", bufs=5) as xs_pool,
            tc.tile_pool(name="ws", bufs=5) as ws_pool,
            tc.tile_pool(name="obuf", bufs=4) as ob_pool,
            tc.tile_pool(name="psum", bufs=6, space="PSUM") as psum_pool,
            tc.tile_pool(name="psumT", bufs=2, space="PSUM") as psumT_pool,
        ):
            ident = const_pool.tile([P, P], mybir.dt.int16)

            # xT u16[p, mb, j, m] = fp8 pair (k=256j+2p, +1) of row m
            xT = xT_pool.tile([P, MB, JB, P], U16)
            # wsgn[p, j, t, n] = sign(w[256j+2p+t, n])
            wsgn = w_pool.tile([P, JB, 2, N_SH], FP8)
            w4d = w.rearrange("(j p t) n -> p j t n", p=P, t=2)
            # x chunk c covers m-blocks 2c, 2c+1: partition p holds rows
            # 256c+p and 256c+128+p
            x3d = x.rearrange("(c two p) k -> p c two k", two=2, p=P)

            xstage = [None] * XC

            def sign_u16(dst, src):
                nc.vector.tensor_scalar(
                    dst, src, AND_MASK, OR_MASK,
                    mybir.AluOpType.bitwise_and, mybir.AluOpType.bitwise_or)

            def load_w(j):
                wr = ws_pool.tile([P, 2, N_SH], FP8, tag="wr")
                nc.gpsimd.dma_start(out=wr[:], in_=w4d[:, j])
                sign_u16(wsgn[:, j, :, :].bitcast(U16), wr[:].bitcast(U16))

            def load_x_raw(c):
                # raw staging for PE-transposed chunks (sign happens at the
                # psum eviction)
                xs = xs_pool.tile([P, 2, K], FP8, tag="xs")
                nc.gpsimd.dma_start(out=xs[:], in_=x3d[:, c])
                xstage[c] = xs

            def load_x_raw_half(c, half):
                if half == 0:
                    xstage[c] = xs_pool.tile([P, 2, K], FP8, tag="xs", name="xsh")
                nc.gpsimd.dma_start(
                    out=xstage[c][:, half, :], in_=x3d[:, c, half, :])

            def sign_x(c):
                # in-place sign of a raw-staged chunk (DVE); emitted in need
                # order so it can never block a due psum eviction
                sign_u16(xstage[c][:].bitcast(U16), xstage[c][:].bitcast(U16))

            def pe_transpose_half(c, half):
                # One m-block (16 u16-blocks) of chunk c through the PE in
                # two TGRP groups; sign is fused into the DVE eviction.
                # The PE transpose runs on BF16 *views* of the u16 pair
                # data: transpose mode is pure routing and bit-preserving
                # (verified on HW for all 65536 patterns), and bf16 is a
                # compiler-accepted PE dtype while u16 is not.  The psum
                # tiles are F32-shaped so the same pool doubles as warmup
                # accumulator space.
                xu = xstage[c][:].bitcast(mybir.dt.bfloat16)   # [P, 2, 2048]
                mbi = 2 * c + half
                for g in range(2):
                    pt = psumT_pool.tile([P, 512], F32, tag="pt", name="pt")
                    ptb = pt[:].bitcast(mybir.dt.bfloat16)     # [P, 1024]
                    for i in range(TGRP):
                        b = TGRP * g + i
                        nc.tensor.transpose(
                            ptb[:, i * P:(i + 1) * P],
                            xu[:, half, b * P:(b + 1) * P],
                            ident[:].bitcast(mybir.dt.bfloat16))
                    sign_u16(xT[:, mbi, TGRP * g:TGRP * (g + 1), :],
                             pt[:].bitcast(U16).rearrange(
                                 "p (a b) -> p a b", a=TGRP))

            def transpose_x(c):
                nc.sync.dma_start(
                    out=xT[:, 2 * c:2 * c + 2, :, :],
                    in_=xstage[c][:].bitcast(U16), transpose=True)

            def mm(po, mb, j, nb, start, stop):
                nc.tensor.matmul(
                    po[:], xT[:, mb, j, :].bitcast(FP8),
                    wsgn[:, j, :, nb * 512:(nb + 1) * 512],
                    start=start, stop=stop, perf_mode=DRSW)

            def mm2(po, mb, j, off, wd, start, stop):
                nc.tensor.matmul(
                    po[:, 0:wd], xT[:, mb, j, :].bitcast(FP8),
                    wsgn[:, j, :, off:off + wd],
                    start=start, stop=stop, perf_mode=DRSW)

            # ---- prologue loads: xc0, then the unbroken w stream, xc1 ----
            # Input DMAs after the w stream are pinned (tile_wait_until) to a
            # hand-planned timeline so the scheduler's enforced DMA order
            # matches what is actually achievable at runtime.
            load_w(0)
            load_w(1)
            load_x_raw(0)
            # identity built after the first loads so its gpsimd memset does
            # not delay the w stream's first SWDGE preps
            make_identity(nc, ident)
            for j in range(2, JB):
                load_w(j)
            with tc.tile_wait_until(PINS['xc1']):
                load_x_raw(1)

            # c0 PE transposes: gated only on the xc0 load; run pre-warmup
            pe_transpose_half(0, 0)
            pe_transpose_half(0, 1)

            # ---- warmup: mb0+mb1 j-interleaved, 6 matmul banks:
            # mb0 all 4 chunks, mb1 chunks 0-1; mb1 chunks 2-3 follow
            # nb-major right after.
            po6 = [psum_pool.tile([P, 512], F32, tag="po", name=f"po{i}")
                   for i in range(6)]
            for j in range(JB):
                for nb in range(NB):
                    mm(po6[nb], 0, j, nb, start=(j == 0), stop=(j == JB - 1))
                for nb in range(2):
                    mm(po6[4 + nb], 1, j, nb,
                       start=(j == 0), stop=(j == JB - 1))

            ob0 = ob_pool.tile([P, N_SH], I16, tag="ob")
            for nb in range(NB):
                nc.vector.tensor_copy(
                    out=ob0[:, nb * 512:(nb + 1) * 512], in_=po6[nb][:])
            nc.scalar.dma_start(out=out[0:P, :], in_=ob0[:])

            ob1 = ob_pool.tile([P, N_SH], I16, tag="ob")
            for nb in range(2):
                nc.vector.tensor_copy(
                    out=ob1[:, nb * 512:(nb + 1) * 512], in_=po6[4 + nb][:])
            # mb1 chunks 2-3 (nb-major) while c1 transposes interleave
            for nb in range(2, NB):
                po = psum_pool.tile([P, 512], F32, tag="po", name="po")
                for j in range(JB):
                    mm(po, 1, j, nb, start=(j == 0), stop=(j == JB - 1))
                if nb == 2:
                    pe_transpose_half(1, 0)
                nc.vector.tensor_copy(
                    out=ob1[:, nb * 512:(nb + 1) * 512], in_=po[:])
            nc.scalar.dma_start(out=out[P:2 * P, :], in_=ob1[:])

            # XBAR-transposed chunks: loads stream right after w on Pool;
            # signs/transposes are emitted in need order below
            for c, ms in ((2, PINS['c2']), (3, PINS['c3']), (4, PINS['c4']),
                          (5, PINS['c5']), (6, PINS['c6'])):
                with tc.tile_wait_until(ms):
                    load_x_raw(c)
            sign_x(2)
            with tc.tile_wait_until(PINS['T2']):
                transpose_x(2)
            sign_x(3)
            with tc.tile_wait_until(PINS['T3']):
                transpose_x(3)

            # ---- steady: mb2..15, nb-major, incremental bank recycling ----
            T_PINS = {4: PINS['T4'], 5: PINS['T5'], 6: PINS['T6'],
                      7: PINS['T7']}
            for mb in range(2, MB):
                if mb == 2:
                    pe_transpose_half(1, 1)
                if 2 <= mb <= 4:
                    sign_x(mb + 2)
                    with tc.tile_wait_until(T_PINS[mb + 2]):
                        transpose_x(mb + 2)
                elif mb == 5:
                    with tc.tile_wait_until(PINS['c7']):
                        load_x_raw(7)
                    sign_x(7)
                    with tc.tile_wait_until(T_PINS[7]):
                        transpose_x(7)
                last = (mb == MB - 1)
                ob = ob_pool.tile([P, N_SH], I16, tag="ob")
                # the last m-block tapers its final chunks (384 then 128 wide)
                # so the exposed end-of-program evict+DMA chain is short
                widths = [512, 512, 512, 384, 128] if last else [512] * NB
                off = 0
                for wd in widths:
                    po = psum_pool.tile([P, 512], F32, tag="po", name="po")
                    for j in range(JB):
                        mm2(po, mb, j, off, wd,
                            start=(j == 0), stop=(j == JB - 1))
                    nsl = slice(off, off + wd)
                    nc.vector.tensor_copy(out=ob[:, nsl], in_=po[:, 0:wd])
                    if last:
                        nc.sync.dma_start(
                            out=out[mb * P:(mb + 1) * P, nsl], in_=ob[:, nsl])
                    off += wd
                if not last:
                    nc.scalar.dma_start(
                        out=out[mb * P:(mb + 1) * P, :], in_=ob[:])

    nc.compile()
    return nc


def get_nc():
    global _NC_CACHE
    if _NC_CACHE is None:
        _NC_CACHE = build_nc()
    return _NC_CACHE


def kernel(x: np.ndarray, w: np.ndarray) -> np.ndarray:
    x = np.asarray(x, dtype=np.float32)
    w = np.asarray(w, dtype=np.float32)
    assert x.shape == (M_FULL, K) and w.shape == (K, N_FULL)

    nc = get_nc()
    in_maps = []
    for c in range(N_CORES):
        mi, ni = divmod(c, RN)
        # SwInterleave reads stationary columns in reverse order: pre-reverse
        # x rows within each 128-row block so output rows land in order.
        xs = x[mi * M_SH:(mi + 1) * M_SH, :]
        xs = xs.reshape(MB, P, K)[:, ::-1, :].reshape(M_SH, K)
        in_maps.append({
            "x": np.ascontiguousarray(xs),
            "w": np.ascontiguousarray(w[:, ni * N_SH:(ni + 1) * N_SH]),
        })
    res = run_bass_kernel_spmd(nc, in_maps, list(range(N_CORES)))

    out = np.empty((M_FULL, N_FULL), dtype=np.float32)
    for c in range(N_CORES):
        mi, ni = divmod(c, RN)
        out[mi * M_SH:(mi + 1) * M_SH, ni * N_SH:(ni + 1) * N_SH] = \
            res.results[c]["out"].astype(np.float32)
    return out



# revision 17
# speedup vs baseline: 1.0539x; 1.0539x over previous
"""BinaryDense kernel for Trainium2: out = sign(x) @ sign(w).

Full shapes: x [8192, 4096] f32, w [4096, 4096] f32 -> out [8192, 4096] f32.
Sharding over 8 NeuronCores: x rows split 4 ways, w columns split 2 ways;
each core computes a [2048, 2048] block.  No collectives.

Core ideas (cost-model-driven):
  - fp8e5 cast-loads (SWDGE): IEEE casts preserve the sign BIT (even on
    underflow to +-0) and only the sign bit matters -> input DMA halves.
  - Bitwise sign on uint16 views: (r & 0x8080) | 0x3C3C == +-1.0 fp8e5 in
    both packed bytes.  Single DVE op per 2 elements, exact.
  - Matmuls: fp8 DoubleRowSwInterleave (0.5 cycles/row).  The k-pair-packed
    transposed x IS the interleaved stationary operand; w cast-loads land
    directly in the plane-separated moving layout.  The mode reads
    stationary columns in reverse order, so the host pre-reverses x rows
    within each 128-row block.
  - PSUM f32 accumulation is exact (+-1 products); int16 out; host widens.

Schedule (slab-pipelined): all input DMAs share one serialized device, so
delivery ORDER is everything.  w arrives as four k-complete 512-column
slabs (each slab: 4 quad-j loads), so each completed slab unlocks a full
m-block sweep on the PE instead of capping pre-stream work at the 2
m-blocks PSUM can j-accumulate.  x chunks 0-1 are PE-transposed early
(w-independent PE work during the stream); chunks 2-7 use XBAR DMA
transposes in the post-stream DMA slack.  wsgn is slab-major so quad
loads land contiguous and sign ops are flat in-place 2-D DVE ops.

Queue map: Pool=cast loads | DVE=signs+evictions | SP=XBAR transposes |
ACT=output DMA issue | PE=matmuls + chunk-0/1 transposes.
"""

import numpy as np

import concourse.mybir as mybir
import concourse.tile as tile
from concourse import bacc
from concourse.bass_utils import run_bass_kernel_spmd
from concourse.masks import make_identity

P = 128
N_CORES = 8
RM, RN = 4, 2
M_FULL, K, N_FULL = 8192, 4096, 4096
M_SH, N_SH = M_FULL // RM, N_FULL // RN   # 2048, 2048
MB = M_SH // P           # 16 m-blocks
JB = K // 256            # 16 k-groups (DoubleRow: 2 planes x 128)
NB = N_SH // 512         # 4 psum-width slabs
XC = MB // 2             # 8 x-chunks of 2 m-blocks
TGRP = 8                 # u16 128-blocks per PE-transpose psum group
QJ = 4                   # j's per w quad-load
NQ = JB // QJ            # 4 quad-loads per slab

F32 = mybir.dt.float32
FP8 = mybir.dt.float8e5
U16 = mybir.dt.uint16
I16 = mybir.dt.int16

AND_MASK = 0x8080
OR_MASK = 0x3C3C
DRSW = mybir.MatmulPerfMode.DoubleRowSwInterleave

_NC_CACHE = None


def build_nc():
    nc = bacc.Bacc("TRN2", target_bir_lowering=False, debug=False,
                   num_devices=N_CORES)
    x = nc.dram_tensor("x", [M_SH, K], F32, kind="ExternalInput").ap()
    w = nc.dram_tensor("w", [K, N_SH], F32, kind="ExternalInput").ap()
    out = nc.dram_tensor("out", [M_SH, N_SH], I16, kind="ExternalOutput").ap()

    with tile.TileContext(nc) as tc:
        with (
            tc.tile_pool(name="const", bufs=1) as const_pool,
            tc.tile_pool(name="xT", bufs=1) as xT_pool,
            tc.tile_pool(name="wbin", bufs=1) as w_pool,
            tc.tile_pool(name="xs", bufs=5) as xs_pool,
            tc.tile_pool(name="obuf", bufs=6) as ob_pool,
            tc.tile_pool(name="psum", bufs=5, space="PSUM") as psum_pool,
            tc.tile_pool(name="psumT", bufs=3, space="PSUM") as psumT_pool,
        ):
            ident = const_pool.tile([P, P], mybir.dt.int16)

            # xT u16[p, mb, j, m] = fp8 pair (k=256j+2p, +1) of row m
            xT = xT_pool.tile([P, MB, JB, P], U16)
            # wsgn[p, s, j, t, n] = sign(w[256j+2p+t, 512s+n]) -- slab-major
            # so quad loads land contiguous and signs are flat 2-D in-place.
            wsgn = w_pool.tile([P, NB, JB, 2, 512], FP8)
            w4d = w.rearrange("(j p t) n -> p j t n", p=P, t=2)
            # x chunk c covers m-blocks 2c, 2c+1: partition p holds rows
            # 256c+p and 256c+128+p
            x3d = x.rearrange("(c two p) k -> p c two k", two=2, p=P)

            xstage = [None] * XC

            def sign_u16(dst, src):
                nc.vector.tensor_scalar(
                    dst, src, AND_MASK, OR_MASK,
                    mybir.AluOpType.bitwise_and, mybir.AluOpType.bitwise_or)

            def load_w_plane(s, t, j0=0, j1=JB):
                # 3-D balanced load: [128, j, 512] one t-plane of slab s
                nc.gpsimd.dma_start(
                    out=wsgn[:, s, j0:j1, t, :],
                    in_=w4d[:, j0:j1, t, 512 * s:512 * (s + 1)])

            def sign_w_half(s, jh):
                # j-half block of slab s is contiguous: flat 2-D in-place sign
                v = wsgn[:, s, 8 * jh:8 * (jh + 1), :, :].bitcast(U16)
                flat = v.rearrange("p a t n -> p (a t n)")
                sign_u16(flat, flat)

            def load_x_raw(c):
                xs = xs_pool.tile([P, 2, K], FP8, tag="xs")
                nc.gpsimd.dma_start(out=xs[:], in_=x3d[:, c])
                xstage[c] = xs

            def load_x_raw_half(c, half):
                if half == 0:
                    xstage[c] = xs_pool.tile([P, 2, K], FP8, tag="xs",
                                             name="xsh")
                nc.gpsimd.dma_start(
                    out=xstage[c][:, half, :], in_=x3d[:, c, half, :])

            def sign_x(c):
                sign_u16(xstage[c][:].bitcast(U16), xstage[c][:].bitcast(U16))

            def pe_transpose_half(c, half):
                # One m-block (16 u16-blocks) of chunk c through the PE in
                # two TGRP groups; sign is fused into the DVE eviction.
                # The PE transpose runs on BF16 *views* of the u16 pair
                # data: transpose mode is pure routing and bit-preserving,
                # and bf16 is a compiler-accepted PE dtype while u16 is not.
                xu = xstage[c][:].bitcast(mybir.dt.bfloat16)   # [P, 2, 2048]
                mbi = 2 * c + half
                for g in range(2):
                    pt = psumT_pool.tile([P, 512], F32, tag="pt", name="pt")
                    ptb = pt[:].bitcast(mybir.dt.bfloat16)     # [P, 1024]
                    for i in range(TGRP):
                        b = TGRP * g + i
                        nc.tensor.transpose(
                            ptb[:, i * P:(i + 1) * P],
                            xu[:, half, b * P:(b + 1) * P],
                            ident[:].bitcast(mybir.dt.bfloat16))
                    sign_u16(xT[:, mbi, TGRP * g:TGRP * (g + 1), :],
                             pt[:].bitcast(U16).rearrange(
                                 "p (a b) -> p a b", a=TGRP))

            def transpose_x(c):
                nc.sync.dma_start(
                    out=xT[:, 2 * c:2 * c + 2, :, :],
                    in_=xstage[c][:].bitcast(U16), transpose=True)

            def mm(po, mb, j, s, start, stop, off=0, wd=512):
                nc.tensor.matmul(
                    po[:, 0:wd], xT[:, mb, j, :].bitcast(FP8),
                    wsgn[:, s, j, :, off:off + wd],
                    start=start, stop=stop, perf_mode=DRSW)

            def sign_x_half(c, half):
                v = xstage[c][:, half, :].bitcast(U16)
                sign_u16(v, v)

            def transpose_x_half(c, half):
                nc.sync.dma_start(
                    out=xT[:, 2 * c + half, :, :],
                    in_=xstage[c][:, half, :].bitcast(U16), transpose=True)

            # ---- Pool load stream: x and w interleaved so the PE's first
            # matmul dependency chain (one x half-chunk + one signed w
            # j-half) completes as early as possible, and each w slab's
            # j-half signs land just ahead of PE consumption.
            load_x_raw_half(0, 0)
            make_identity(nc, ident)
            load_w_plane(0, 0, 0, 8)
            load_w_plane(0, 1, 0, 8)
            load_x_raw_half(0, 1)
            load_w_plane(0, 0, 8, JB)
            load_w_plane(0, 1, 8, JB)
            load_x_raw(1)
            for s in range(1, NB):
                for jh in (0, 1):
                    load_w_plane(s, 0, 8 * jh, 8 * (jh + 1))
                    load_w_plane(s, 1, 8 * jh, 8 * (jh + 1))
            load_x_raw_half(2, 0)
            load_x_raw_half(2, 1)

            # ---- PE + DVE emission: transposes fill w-delivery latency.
            # DVE carries signs + transpose evictions in true arrival order;
            # psum evictions run on ACT so they never block a sign.
            ob03 = [ob_pool.tile([P, N_SH], I16, tag="ob", name=f"ob{m}")
                    for m in range(4)]
            po4 = [psum_pool.tile([P, 512], F32, tag="po", name="po4")
                   for _ in range(4)]

            v000 = wsgn[:, 0, 0:8, 0, :].bitcast(U16)
            sign_u16(v000, v000)
            pe_transpose_half(0, 0)
            v001 = wsgn[:, 0, 0:8, 1, :].bitcast(U16)
            sign_u16(v001, v001)
            for j in range(8):
                mm(po4[0], 0, j, 0, start=(j == 0), stop=False)
            pe_transpose_half(0, 1)
            for j in range(8):
                mm(po4[1], 1, j, 0, start=(j == 0), stop=False)
            sign_w_half(0, 1)
            for j in range(8, JB):
                mm(po4[0], 0, j, 0, start=False, stop=(j == JB - 1))
            for j in range(8, JB):
                mm(po4[1], 1, j, 0, start=False, stop=(j == JB - 1))
            pe_transpose_half(1, 0)
            pe_transpose_half(1, 1)
            for mb in (2, 3):
                for j in range(JB):
                    mm(po4[mb], mb, j, 0, start=(j == 0), stop=(j == JB - 1))
            for s in range(1, NB):
                sign_w_half(s, 0)
                sign_w_half(s, 1)
            sign_x_half(2, 0)
            sign_x_half(2, 1)
            transpose_x_half(2, 0)
            transpose_x_half(2, 1)

            # slab 0 evicts + slabs 1-3 (jh-major, mb inner: consumption
            # tracks the j-half sign granularity)
            for s in range(NB):
                if s > 0:
                    po4 = [psum_pool.tile([P, 512], F32, tag="po",
                                          name="po4")
                           for _ in range(4)]
                    for jh in (0, 1):
                        for mb in range(4):
                            for j in range(8 * jh, 8 * (jh + 1)):
                                mm(po4[mb], mb, j, s,
                                   start=(j == 0), stop=(j == JB - 1))
                nsl = slice(512 * s, 512 * (s + 1))
                for mb in range(4):
                    nc.scalar.copy(out=ob03[mb][:, nsl], in_=po4[mb][:])

            # stores issue from the Pool/SWDGE queue: ACT stays a pure
            # eviction engine (a store's DMA-queue-depth wait on ACT.SEQ
            # would head-of-line block later evictions and stall the PE
            # on psum recycling)
            for m in range(4):
                nc.gpsimd.dma_start(out=out[m * P:(m + 1) * P, :],
                                    in_=ob03[m][:])

            # ---- steady: mb4..15; half-chunk x chains, each cast-load
            # (SWDGE-only) released one m-block-sweep ahead of need: a tiny
            # Pool copy reads the LAST column of m-block nmb-3's ob tile
            # (written by its final evict) and writes into the load's own
            # destination region, so the load has a WAW dependency on the
            # gate and the serialized DMA device serves the XBAR
            # transposes the PE needs first.
            obs = {m: ob03[m] for m in range(4)}
            for mb in range(4, MB):
                nmb = mb + 2
                if nmb < MB:
                    c, half = divmod(nmb, 2)
                    if half == 0:
                        xstage[c] = xs_pool.tile([P, 2, K], FP8, tag="xs",
                                                 name="xsg")
                    gmb = nmb - 4 if nmb <= 7 else nmb - 3
                    nc.gpsimd.tensor_copy(
                        out=xstage[c][:, half, 0:2],
                        in_=obs[gmb][:, N_SH - 2:N_SH])
                    nc.gpsimd.dma_start(
                        out=xstage[c][:, half, :], in_=x3d[:, c, half, :])
                    sign_x_half(c, half)
                    transpose_x_half(c, half)
                last = (mb == MB - 1)
                ob = ob_pool.tile([P, N_SH], I16, tag="ob")
                obs[mb] = ob
                # the last m-block tapers its final chunks so the exposed
                # end-of-program evict+DMA chain is short
                widths = [(0, 512), (1, 512), (2, 512), (3, 448), (3, 64)] \
                    if last else [(s, 512) for s in range(NB)]
                off_in_s = 0
                prev_s = 0
                for s, wd in widths:
                    if s != prev_s:
                        off_in_s = 0
                        prev_s = s
                    po = psum_pool.tile([P, 512], F32, tag="po", name="po")
                    for j in range(JB):
                        mm(po, mb, j, s, start=(j == 0), stop=(j == JB - 1),
                           off=off_in_s, wd=wd)
                    nsl = slice(512 * s + off_in_s, 512 * s + off_in_s + wd)
                    nc.scalar.copy(out=ob[:, nsl], in_=po[:, 0:wd])
                    if last:
                        # overlap the two final stores on different queues
                        eng = nc.scalar if wd == 448 else nc.sync
                        eng.dma_start(
                            out=out[mb * P:(mb + 1) * P, nsl], in_=ob[:, nsl])
                    off_in_s += wd
                if not last:
                    nc.gpsimd.dma_start(
                        out=out[mb * P:(mb + 1) * P, :], in_=ob[:])

    nc.compile()
    return nc


def get_nc():
    global _NC_CACHE
    if _NC_CACHE is None:
        _NC_CACHE = build_nc()
    return _NC_CACHE


def kernel(x: np.ndarray, w: np.ndarray) -> np.ndarray:
    x = np.asarray(x, dtype=np.float32)
    w = np.asarray(w, dtype=np.float32)
    assert x.shape == (M_FULL, K) and w.shape == (K, N_FULL)

    nc = get_nc()
    in_maps = []
    for c in range(N_CORES):
        mi, ni = divmod(c, RN)
        # SwInterleave reads stationary columns in reverse order: pre-reverse
        # x rows within each 128-row block so output rows land in order.
        xs = x[mi * M_SH:(mi + 1) * M_SH, :]
        xs = xs.reshape(MB, P, K)[:, ::-1, :].reshape(M_SH, K)
        in_maps.append({
            "x": np.ascontiguousarray(xs),
            "w": np.ascontiguousarray(w[:, ni * N_SH:(ni + 1) * N_SH]),
        })
    res = run_bass_kernel_spmd(nc, in_maps, list(range(N_CORES)))

    out = np.empty((M_FULL, N_FULL), dtype=np.float32)
    for c in range(N_CORES):
        mi, ni = divmod(c, RN)
        out[mi * M_SH:(mi + 1) * M_SH, ni * N_SH:(ni + 1) * N_SH] = \
            res.results[c]["out"].astype(np.float32)
    return out


# revision 32
# speedup vs baseline: 1.0548x; 1.0009x over previous
"""BinaryDense kernel for Trainium2: out = sign(x) @ sign(w).

Full shapes: x [8192, 4096] f32, w [4096, 4096] f32 -> out [8192, 4096] f32.
Sharding over 8 NeuronCores: x rows split 4 ways, w columns split 2 ways;
each core computes a [2048, 2048] block.  No collectives.

Core ideas (cost-model-driven):
  - fp8e5 cast-loads (SWDGE): IEEE casts preserve the sign BIT (even on
    underflow to +-0) and only the sign bit matters -> input DMA halves.
  - Bitwise sign on uint16 views: (r & 0x8080) | 0x3C3C == +-1.0 fp8e5 in
    both packed bytes.  Single DVE op per 2 elements, exact.
  - Matmuls: fp8 DoubleRowSwInterleave (0.5 cycles/row).  The k-pair-packed
    transposed x IS the interleaved stationary operand; w cast-loads land
    directly in the plane-separated moving layout.  The mode reads
    stationary columns in reverse order, so the host pre-reverses x rows
    within each 128-row block.
  - PSUM f32 accumulation is exact (+-1 products); int16 out; host widens.

Schedule (slab-pipelined): all input DMAs share one serialized device, so
delivery ORDER is everything.  w arrives as four k-complete 512-column
slabs (each slab: 4 quad-j loads), so each completed slab unlocks a full
m-block sweep on the PE instead of capping pre-stream work at the 2
m-blocks PSUM can j-accumulate.  x chunks 0-1 are PE-transposed early
(w-independent PE work during the stream); chunks 2-7 use XBAR DMA
transposes in the post-stream DMA slack.  wsgn is slab-major so quad
loads land contiguous and sign ops are flat in-place 2-D DVE ops.

Queue map: Pool=cast loads | DVE=signs+evictions | SP=XBAR transposes |
ACT=output DMA issue | PE=matmuls + chunk-0/1 transposes.
"""

import numpy as np

import concourse.mybir as mybir
import concourse.tile as tile
from concourse import bacc
from concourse.bass_utils import run_bass_kernel_spmd
from concourse.masks import make_identity

P = 128
N_CORES = 8
RM, RN = 4, 2
M_FULL, K, N_FULL = 8192, 4096, 4096
M_SH, N_SH = M_FULL // RM, N_FULL // RN   # 2048, 2048
MB = M_SH // P           # 16 m-blocks
JB = K // 256            # 16 k-groups (DoubleRow: 2 planes x 128)
NB = N_SH // 512         # 4 psum-width slabs
XC = MB // 2             # 8 x-chunks of 2 m-blocks
TGRP = 8                 # u16 128-blocks per PE-transpose psum group
QJ = 4                   # j's per w quad-load
NQ = JB // QJ            # 4 quad-loads per slab

F32 = mybir.dt.float32
FP8 = mybir.dt.float8e5
U16 = mybir.dt.uint16
I16 = mybir.dt.int16

AND_MASK = 0x8080
OR_MASK = 0x3C3C
DRSW = mybir.MatmulPerfMode.DoubleRowSwInterleave

_NC_CACHE = None


def build_nc():
    nc = bacc.Bacc("TRN2", target_bir_lowering=False, debug=False,
                   num_devices=N_CORES)
    x = nc.dram_tensor("x", [M_SH, K], F32, kind="ExternalInput").ap()
    w = nc.dram_tensor("w", [K, N_SH], F32, kind="ExternalInput").ap()
    out = nc.dram_tensor("out", [M_SH, N_SH], I16, kind="ExternalOutput").ap()

    with tile.TileContext(nc) as tc:
        with (
            tc.tile_pool(name="const", bufs=1) as const_pool,
            tc.tile_pool(name="xT", bufs=1) as xT_pool,
            tc.tile_pool(name="wbin", bufs=1) as w_pool,
            tc.tile_pool(name="xs", bufs=5) as xs_pool,
            tc.tile_pool(name="obuf", bufs=6) as ob_pool,
            tc.tile_pool(name="psum", bufs=6, space="PSUM") as psum_pool,
            tc.tile_pool(name="psumT", bufs=2, space="PSUM") as psumT_pool,
        ):
            ident = const_pool.tile([P, P], mybir.dt.int16)

            # xT u16[p, mb, j, m] = fp8 pair (k=256j+2p, +1) of row m
            xT = xT_pool.tile([P, MB, JB, P], U16)
            # wsgn[p, s, j, t, n] = sign(w[256j+2p+t, 512s+n]) -- slab-major
            # so quad loads land contiguous and signs are flat 2-D in-place.
            wsgn = w_pool.tile([P, NB, JB, 2, 512], FP8)
            w4d = w.rearrange("(j p t) n -> p j t n", p=P, t=2)
            # x chunk c covers m-blocks 2c, 2c+1: partition p holds rows
            # 256c+p and 256c+128+p
            x3d = x.rearrange("(c two p) k -> p c two k", two=2, p=P)

            xstage = [None] * XC

            def sign_u16(dst, src):
                nc.vector.tensor_scalar(
                    dst, src, AND_MASK, OR_MASK,
                    mybir.AluOpType.bitwise_and, mybir.AluOpType.bitwise_or)

            def load_w_plane(s, t, j0=0, j1=JB):
                # 3-D balanced load: [128, j, 512] one t-plane of slab s
                nc.gpsimd.dma_start(
                    out=wsgn[:, s, j0:j1, t, :],
                    in_=w4d[:, j0:j1, t, 512 * s:512 * (s + 1)])

            def sign_w_half(s, jh):
                # j-half block of slab s is contiguous: flat 2-D in-place sign
                v = wsgn[:, s, 8 * jh:8 * (jh + 1), :, :].bitcast(U16)
                flat = v.rearrange("p a t n -> p (a t n)")
                sign_u16(flat, flat)

            def load_x_raw(c):
                xs = xs_pool.tile([P, 2, K], FP8, tag="xs")
                nc.gpsimd.dma_start(out=xs[:], in_=x3d[:, c])
                xstage[c] = xs

            def load_x_raw_half(c, half):
                if half == 0:
                    xstage[c] = xs_pool.tile([P, 2, K], FP8, tag="xs",
                                             name="xsh")
                nc.gpsimd.dma_start(
                    out=xstage[c][:, half, :], in_=x3d[:, c, half, :])

            def sign_x(c):
                sign_u16(xstage[c][:].bitcast(U16), xstage[c][:].bitcast(U16))

            def pe_transpose_group(c, half, g):
                xu = xstage[c][:].bitcast(mybir.dt.bfloat16)
                mbi = 2 * c + half
                pt = psumT_pool.tile([P, 512], F32, tag="pt", name="pt")
                ptb = pt[:].bitcast(mybir.dt.bfloat16)
                for i in range(TGRP):
                    b = TGRP * g + i
                    nc.tensor.transpose(
                        ptb[:, i * P:(i + 1) * P],
                        xu[:, half, b * P:(b + 1) * P],
                        ident[:].bitcast(mybir.dt.bfloat16))
                sign_u16(xT[:, mbi, TGRP * g:TGRP * (g + 1), :],
                         pt[:].bitcast(U16).rearrange(
                             "p (a b) -> p a b", a=TGRP))

            def pe_transpose_half(c, half):
                # One m-block (16 u16-blocks) of chunk c through the PE in
                # two TGRP groups; sign is fused into the DVE eviction.
                # The PE transpose runs on BF16 *views* of the u16 pair
                # data: transpose mode is pure routing and bit-preserving,
                # and bf16 is a compiler-accepted PE dtype while u16 is not.
                xu = xstage[c][:].bitcast(mybir.dt.bfloat16)   # [P, 2, 2048]
                mbi = 2 * c + half
                for g in range(2):
                    pt = psumT_pool.tile([P, 512], F32, tag="pt", name="pt")
                    ptb = pt[:].bitcast(mybir.dt.bfloat16)     # [P, 1024]
                    for i in range(TGRP):
                        b = TGRP * g + i
                        nc.tensor.transpose(
                            ptb[:, i * P:(i + 1) * P],
                            xu[:, half, b * P:(b + 1) * P],
                            ident[:].bitcast(mybir.dt.bfloat16))
                    sign_u16(xT[:, mbi, TGRP * g:TGRP * (g + 1), :],
                             pt[:].bitcast(U16).rearrange(
                                 "p (a b) -> p a b", a=TGRP))

            def transpose_x(c):
                nc.sync.dma_start(
                    out=xT[:, 2 * c:2 * c + 2, :, :],
                    in_=xstage[c][:].bitcast(U16), transpose=True)

            def mm(po, mb, j, s, start, stop, off=0, wd=512):
                nc.tensor.matmul(
                    po[:, 0:wd], xT[:, mb, j, :].bitcast(FP8),
                    wsgn[:, s, j, :, off:off + wd],
                    start=start, stop=stop, perf_mode=DRSW)

            def sign_x_half(c, half):
                v = xstage[c][:, half, :].bitcast(U16)
                sign_u16(v, v)

            def transpose_x_half(c, half):
                nc.sync.dma_start(
                    out=xT[:, 2 * c + half, :, :],
                    in_=xstage[c][:, half, :].bitcast(U16), transpose=True)

            # ---- Pool load stream: x and w interleaved so the PE's first
            # matmul dependency chain (one x half-chunk + one signed w
            # j-half) completes as early as possible, and each w slab's
            # j-half signs land just ahead of PE consumption.
            load_x_raw_half(0, 0)
            make_identity(nc, ident)
            load_w_plane(0, 0, 0, 8)
            load_w_plane(0, 1, 0, 8)
            load_x_raw_half(0, 1)
            load_x_raw(1)
            load_w_plane(0, 0, 8, JB)
            load_w_plane(0, 1, 8, JB)
            for s in range(1, NB):
                for jh in (0, 1):
                    load_w_plane(s, 0, 8 * jh, 8 * (jh + 1))
                    load_w_plane(s, 1, 8 * jh, 8 * (jh + 1))
            load_x_raw_half(2, 0)
            load_x_raw_half(2, 1)

            # ---- PE + DVE emission: transposes fill w-delivery latency.
            # DVE carries signs + transpose evictions in true arrival order;
            # psum evictions run on ACT so they never block a sign.
            ob03 = [ob_pool.tile([P, N_SH], I16, tag="ob", name=f"ob{m}")
                    for m in range(4)]
            po4 = [psum_pool.tile([P, 512], F32, tag="po", name="po4")
                   for _ in range(4)]
            gate_po = {}

            v000 = wsgn[:, 0, 0:8, 0, :].bitcast(U16)
            sign_u16(v000, v000)
            with tc.high_priority(offset=60):
                pe_transpose_group(0, 0, 0)
                pe_transpose_group(0, 0, 1)
            v001 = wsgn[:, 0, 0:8, 1, :].bitcast(U16)
            sign_u16(v001, v001)
            for j in range(8):
                mm(po4[0], 0, j, 0, start=(j == 0), stop=False)
            pe_transpose_half(0, 1)
            for j in range(8):
                mm(po4[1], 1, j, 0, start=(j == 0), stop=False)
            pe_transpose_half(1, 0)
            pe_transpose_half(1, 1)
            for mb in (2, 3):
                for j in range(8):
                    mm(po4[mb], mb, j, 0, start=(j == 0), stop=False)
            sign_w_half(0, 1)
            for mb in range(4):
                for j in range(8, JB):
                    mm(po4[mb], mb, j, 0, start=False, stop=(j == JB - 1))
            for s in range(1, NB):
                sign_w_half(s, 0)
                sign_w_half(s, 1)
            sign_x_half(2, 0)
            sign_x_half(2, 1)
            transpose_x_half(2, 0)
            transpose_x_half(2, 1)

            # slab 0 evicts + slabs 1-3 (jh-major, mb inner: consumption
            # tracks the j-half sign granularity)
            for s in range(NB):
                if s > 0:
                    po4 = [psum_pool.tile([P, 512], F32, tag="po",
                                          name="po4")
                           for _ in range(4)]
                    for jh in (0, 1):
                        for mb in range(4):
                            for j in range(8 * jh, 8 * (jh + 1)):
                                mm(po4[mb], mb, j, s,
                                   start=(j == 0), stop=(j == JB - 1))
                nsl = slice(512 * s, 512 * (s + 1))
                for mb in range(4):
                    if s == NB - 1:
                        gate_po[mb] = po4[mb]
                    nc.scalar.copy(out=ob03[mb][:, nsl], in_=po4[mb][:])

            for m in range(4):
                nc.scalar.dma_start(out=out[m * P:(m + 1) * P, :],
                                    in_=ob03[m][:])

            # ---- steady: mb4..15; half-chunk x chains, each cast-load
            # (SWDGE-only) released one m-block-sweep ahead of need: a tiny
            # Pool copy reads the LAST column of m-block nmb-3's ob tile
            # (written by its final evict) and writes into the load's own
            # destination region, so the load has a WAW dependency on the
            # gate and the serialized DMA device serves the XBAR
            # transposes the PE needs first.
            obs = {m: ob03[m] for m in range(4)}
            MB_US = 6.85
            for mb in range(4, MB):
                nmb = mb + 2
                if nmb < MB:
                    c, half = divmod(nmb, 2)
                    if half == 0:
                        xstage[c] = xs_pool.tile([P, 2, K], FP8, tag="xs",
                                                 name="xsg")
                    gmb = nmb - 4 if nmb <= 7 else nmb - 3
                    nc.gpsimd.tensor_copy(
                        out=xstage[c][:, half, 0:2],
                        in_=obs[gmb][:, N_SH - 2:N_SH])
                    nc.gpsimd.dma_start(
                        out=xstage[c][:, half, :], in_=x3d[:, c, half, :])
                    sign_x_half(c, half)
                    transpose_x_half(c, half)
                last = (mb == MB - 1)
                ob = ob_pool.tile([P, N_SH], I16, tag="ob")
                obs[mb] = ob
                # the last m-block tapers its final chunks so the exposed
                # end-of-program evict+DMA chain is short
                widths = [(0, 512), (1, 512), (2, 512), (3, 512)] \
                    if last else [(s, 512) for s in range(NB)]
                off_in_s = 0
                prev_s = 0
                for s, wd in widths:
                    if s != prev_s:
                        off_in_s = 0
                        prev_s = s
                    po = psum_pool.tile([P, 512], F32, tag="po", name="po")
                    for j in range(JB):
                        mm(po, mb, j, s, start=(j == 0), stop=(j == JB - 1),
                           off=off_in_s, wd=wd)
                    gate_po[mb] = po
                    nsl = slice(512 * s + off_in_s, 512 * s + off_in_s + wd)
                    nc.scalar.copy(out=ob[:, nsl], in_=po[:, 0:wd])
                    if last:
                        # overlap the two final stores on different queues
                        eng = nc.scalar if wd == 448 else nc.sync
                        eng.dma_start(
                            out=out[mb * P:(mb + 1) * P, nsl], in_=ob[:, nsl])
                    off_in_s += wd
                if not last:
                    nc.scalar.dma_start(
                        out=out[mb * P:(mb + 1) * P, :], in_=ob[:])

    nc.compile()
    return nc


def get_nc():
    global _NC_CACHE
    if _NC_CACHE is None:
        _NC_CACHE = build_nc()
    return _NC_CACHE


def kernel(x: np.ndarray, w: np.ndarray) -> np.ndarray:
    x = np.asarray(x, dtype=np.float32)
    w = np.asarray(w, dtype=np.float32)
    assert x.shape == (M_FULL, K) and w.shape == (K, N_FULL)

    nc = get_nc()
    in_maps = []
    for c in range(N_CORES):
        mi, ni = divmod(c, RN)
        # SwInterleave reads stationary columns in reverse order: pre-reverse
        # x rows within each 128-row block so output rows land in order.
        xs = x[mi * M_SH:(mi + 1) * M_SH, :]
        xs = xs.reshape(MB, P, K)[:, ::-1, :].reshape(M_SH, K)
        in_maps.append({
            "x": np.ascontiguousarray(xs),
            "w": np.ascontiguousarray(w[:, ni * N_SH:(ni + 1) * N_SH]),
        })
    res = run_bass_kernel_spmd(nc, in_maps, list(range(N_CORES)))

    out = np.empty((M_FULL, N_FULL), dtype=np.float32)
    for c in range(N_CORES):
        mi, ni = divmod(c, RN)
        out[mi * M_SH:(mi + 1) * M_SH, ni * N_SH:(ni + 1) * N_SH] = \
            res.results[c]["out"].astype(np.float32)
    return out


# revision 40
# speedup vs baseline: 1.0582x; 1.0032x over previous
"""BinaryDense kernel for Trainium2: out = sign(x) @ sign(w).

Full shapes: x [8192, 4096] f32, w [4096, 4096] f32 -> out [8192, 4096] f32.
Sharding over 8 NeuronCores: x rows split 4 ways, w columns split 2 ways;
each core computes a [2048, 2048] block.  No collectives.

Core ideas (cost-model-driven):
  - fp8e5 cast-loads (SWDGE): IEEE casts preserve the sign BIT (even on
    underflow to +-0) and only the sign bit matters -> input DMA halves.
  - Bitwise sign on uint16 views: (r & 0x8080) | 0x3C3C == +-1.0 fp8e5 in
    both packed bytes.  Single DVE op per 2 elements, exact.
  - Matmuls: fp8 DoubleRowSwInterleave (0.5 cycles/row).  The k-pair-packed
    transposed x IS the interleaved stationary operand; w cast-loads land
    directly in the plane-separated moving layout.  The mode reads
    stationary columns in reverse order, so the host pre-reverses x rows
    within each 128-row block.
  - PSUM f32 accumulation is exact (+-1 products); int16 out; host widens.

Schedule (slab-pipelined): all input DMAs share one serialized device, so
delivery ORDER is everything.  w arrives as four k-complete 512-column
slabs (each slab: 4 quad-j loads), so each completed slab unlocks a full
m-block sweep on the PE instead of capping pre-stream work at the 2
m-blocks PSUM can j-accumulate.  x chunks 0-1 are PE-transposed early
(w-independent PE work during the stream); chunks 2-7 use XBAR DMA
transposes in the post-stream DMA slack.  wsgn is slab-major so quad
loads land contiguous and sign ops are flat in-place 2-D DVE ops.

Queue map: Pool=cast loads | DVE=signs+evictions | SP=XBAR transposes |
ACT=output DMA issue | PE=matmuls + chunk-0/1 transposes.
"""

import numpy as np

import concourse.mybir as mybir
import concourse.tile as tile
from concourse import bacc
from concourse.bass_utils import run_bass_kernel_spmd
from concourse.masks import make_identity

P = 128
N_CORES = 8
RM, RN = 4, 2
M_FULL, K, N_FULL = 8192, 4096, 4096
M_SH, N_SH = M_FULL // RM, N_FULL // RN   # 2048, 2048
MB = M_SH // P           # 16 m-blocks
JB = K // 256            # 16 k-groups (DoubleRow: 2 planes x 128)
NB = N_SH // 512         # 4 psum-width slabs
XC = MB // 2             # 8 x-chunks of 2 m-blocks
TGRP = 8                 # u16 128-blocks per PE-transpose psum group
QJ = 4                   # j's per w quad-load
NQ = JB // QJ            # 4 quad-loads per slab

F32 = mybir.dt.float32
FP8 = mybir.dt.float8e5
U16 = mybir.dt.uint16
I16 = mybir.dt.int16

AND_MASK = 0x8080
OR_MASK = 0x3C3C
DRSW = mybir.MatmulPerfMode.DoubleRowSwInterleave

_NC_CACHE = None


def build_nc():
    nc = bacc.Bacc("TRN2", target_bir_lowering=False, debug=False,
                   num_devices=N_CORES)
    x = nc.dram_tensor("x", [M_SH, K], F32, kind="ExternalInput").ap()
    w = nc.dram_tensor("w", [K, N_SH], F32, kind="ExternalInput").ap()
    out = nc.dram_tensor("out", [M_SH, N_SH], I16, kind="ExternalOutput").ap()

    with tile.TileContext(nc) as tc:
        with (
            tc.tile_pool(name="const", bufs=1) as const_pool,
            tc.tile_pool(name="xT", bufs=1) as xT_pool,
            tc.tile_pool(name="wbin", bufs=1) as w_pool,
            tc.tile_pool(name="xs", bufs=5) as xs_pool,
            tc.tile_pool(name="obuf", bufs=6) as ob_pool,
            tc.tile_pool(name="psum", bufs=6, space="PSUM") as psum_pool,
            tc.tile_pool(name="psumT", bufs=2, space="PSUM") as psumT_pool,
        ):
            ident = const_pool.tile([P, P], mybir.dt.int16)

            # xT u16[p, mb, j, m] = fp8 pair (k=256j+2p, +1) of row m
            xT = xT_pool.tile([P, MB, JB, P], U16)
            # wsgn[p, s, j, t, n] = sign(w[256j+2p+t, 512s+n]) -- slab-major
            # so quad loads land contiguous and signs are flat 2-D in-place.
            wsgn = w_pool.tile([P, NB, JB, 2, 512], FP8)
            w4d = w.rearrange("(j p t) n -> p j t n", p=P, t=2)
            # x chunk c covers m-blocks 2c, 2c+1: partition p holds rows
            # 256c+p and 256c+128+p
            x3d = x.rearrange("(c two p) k -> p c two k", two=2, p=P)

            xstage = [None] * XC

            def sign_u16(dst, src):
                nc.vector.tensor_scalar(
                    dst, src, AND_MASK, OR_MASK,
                    mybir.AluOpType.bitwise_and, mybir.AluOpType.bitwise_or)

            def load_w_plane(s, t, j0=0, j1=JB):
                # 3-D balanced load: [128, j, 512] one t-plane of slab s
                nc.gpsimd.dma_start(
                    out=wsgn[:, s, j0:j1, t, :],
                    in_=w4d[:, j0:j1, t, 512 * s:512 * (s + 1)])

            def sign_w_half(s, jh):
                # j-half block of slab s is contiguous: flat 2-D in-place sign
                v = wsgn[:, s, 8 * jh:8 * (jh + 1), :, :].bitcast(U16)
                flat = v.rearrange("p a t n -> p (a t n)")
                sign_u16(flat, flat)

            def load_x_raw(c):
                xs = xs_pool.tile([P, 2, K], FP8, tag="xs")
                nc.gpsimd.dma_start(out=xs[:], in_=x3d[:, c])
                xstage[c] = xs

            def load_x_raw_half(c, half):
                if half == 0:
                    xstage[c] = xs_pool.tile([P, 2, K], FP8, tag="xs",
                                             name="xsh")
                nc.gpsimd.dma_start(
                    out=xstage[c][:, half, :], in_=x3d[:, c, half, :])

            def sign_x(c):
                sign_u16(xstage[c][:].bitcast(U16), xstage[c][:].bitcast(U16))

            def pe_transpose_group(c, half, g):
                xu = xstage[c][:].bitcast(mybir.dt.bfloat16)
                mbi = 2 * c + half
                pt = psumT_pool.tile([P, 512], F32, tag="pt", name="pt")
                ptb = pt[:].bitcast(mybir.dt.bfloat16)
                for i in range(TGRP):
                    b = TGRP * g + i
                    nc.tensor.transpose(
                        ptb[:, i * P:(i + 1) * P],
                        xu[:, half, b * P:(b + 1) * P],
                        ident[:].bitcast(mybir.dt.bfloat16))
                sign_u16(xT[:, mbi, TGRP * g:TGRP * (g + 1), :],
                         pt[:].bitcast(U16).rearrange(
                             "p (a b) -> p a b", a=TGRP))

            def pe_transpose_half(c, half):
                # One m-block (16 u16-blocks) of chunk c through the PE in
                # two TGRP groups; sign is fused into the DVE eviction.
                # The PE transpose runs on BF16 *views* of the u16 pair
                # data: transpose mode is pure routing and bit-preserving,
                # and bf16 is a compiler-accepted PE dtype while u16 is not.
                xu = xstage[c][:].bitcast(mybir.dt.bfloat16)   # [P, 2, 2048]
                mbi = 2 * c + half
                for g in range(2):
                    pt = psumT_pool.tile([P, 512], F32, tag="pt", name="pt")
                    ptb = pt[:].bitcast(mybir.dt.bfloat16)     # [P, 1024]
                    for i in range(TGRP):
                        b = TGRP * g + i
                        nc.tensor.transpose(
                            ptb[:, i * P:(i + 1) * P],
                            xu[:, half, b * P:(b + 1) * P],
                            ident[:].bitcast(mybir.dt.bfloat16))
                    sign_u16(xT[:, mbi, TGRP * g:TGRP * (g + 1), :],
                             pt[:].bitcast(U16).rearrange(
                                 "p (a b) -> p a b", a=TGRP))

            def transpose_x(c):
                nc.sync.dma_start(
                    out=xT[:, 2 * c:2 * c + 2, :, :],
                    in_=xstage[c][:].bitcast(U16), transpose=True)

            def mm(po, mb, j, s, start, stop, off=0, wd=512):
                nc.tensor.matmul(
                    po[:, 0:wd], xT[:, mb, j, :].bitcast(FP8),
                    wsgn[:, s, j, :, off:off + wd],
                    start=start, stop=stop, perf_mode=DRSW)

            def sign_x_half(c, half):
                v = xstage[c][:, half, :].bitcast(U16)
                sign_u16(v, v)

            def transpose_x_half(c, half):
                nc.sync.dma_start(
                    out=xT[:, 2 * c + half, :, :],
                    in_=xstage[c][:, half, :].bitcast(U16), transpose=True)

            # ---- Pool load stream: x and w interleaved so the PE's first
            # matmul dependency chain (one x half-chunk + one signed w
            # j-half) completes as early as possible, and each w slab's
            # j-half signs land just ahead of PE consumption.
            load_x_raw_half(0, 0)
            make_identity(nc, ident)
            load_w_plane(0, 0, 0, 8)
            load_w_plane(0, 1, 0, 8)
            load_x_raw_half(0, 1)
            load_x_raw(1)
            load_w_plane(0, 0, 8, JB)
            load_w_plane(0, 1, 8, JB)
            for s in range(1, NB):
                for jh in (0, 1):
                    load_w_plane(s, 0, 8 * jh, 8 * (jh + 1))
                    load_w_plane(s, 1, 8 * jh, 8 * (jh + 1))
            load_x_raw_half(2, 0)
            load_x_raw_half(2, 1)

            # ---- PE + DVE emission: transposes fill w-delivery latency.
            # DVE carries signs + transpose evictions in true arrival order;
            # psum evictions run on ACT so they never block a sign.
            ob03 = [ob_pool.tile([P, N_SH], I16, tag="ob", name=f"ob{m}")
                    for m in range(4)]
            po4 = [psum_pool.tile([P, 512], F32, tag="po", name="po4")
                   for _ in range(4)]
            gate_po = {}

            v000 = wsgn[:, 0, 0:8, 0, :].bitcast(U16)
            sign_u16(v000, v000)
            pe_transpose_group(0, 0, 0)
            pe_transpose_group(0, 0, 1)
            v001 = wsgn[:, 0, 0:8, 1, :].bitcast(U16)
            sign_u16(v001, v001)
            for j in range(8):
                mm(po4[0], 0, j, 0, start=(j == 0), stop=False)
            pe_transpose_half(0, 1)
            for j in range(8):
                mm(po4[1], 1, j, 0, start=(j == 0), stop=False)
            pe_transpose_half(1, 0)
            pe_transpose_half(1, 1)
            for mb in (2, 3):
                for j in range(8):
                    mm(po4[mb], mb, j, 0, start=(j == 0), stop=False)
            sign_w_half(0, 1)
            for mb in range(4):
                for j in range(8, JB):
                    mm(po4[mb], mb, j, 0, start=False, stop=(j == JB - 1))
            for s in range(1, NB):
                sign_w_half(s, 0)
                sign_w_half(s, 1)
            sign_x_half(2, 0)
            sign_x_half(2, 1)
            transpose_x_half(2, 0)
            transpose_x_half(2, 1)

            # slab 0 evicts + slabs 1-3 (jh-major, mb inner: consumption
            # tracks the j-half sign granularity)
            for s in range(NB):
                if s > 0:
                    po4 = [psum_pool.tile([P, 512], F32, tag="po",
                                          name="po4")
                           for _ in range(4)]
                    for jh in (0, 1):
                        for mb in range(4):
                            for j in range(8 * jh, 8 * (jh + 1)):
                                mm(po4[mb], mb, j, s,
                                   start=(j == 0), stop=(j == JB - 1))
                nsl = slice(512 * s, 512 * (s + 1))
                for mb in range(4):
                    if s == NB - 1:
                        gate_po[mb] = po4[mb]
                    nc.scalar.copy(out=ob03[mb][:, nsl], in_=po4[mb][:])

            for m in range(4):
                nc.scalar.dma_start(out=out[m * P:(m + 1) * P, :],
                                    in_=ob03[m][:])

            # ---- steady: mb4..15; half-chunk x chains, each cast-load
            # (SWDGE-only) released one m-block-sweep ahead of need: a tiny
            # Pool copy reads the LAST column of m-block nmb-3's ob tile
            # (written by its final evict) and writes into the load's own
            # destination region, so the load has a WAW dependency on the
            # gate and the serialized DMA device serves the XBAR
            # transposes the PE needs first.
            obs = {m: ob03[m] for m in range(4)}
            MB_US = 6.85
            for mb in range(4, MB):
                nmb = mb + 2
                if nmb < MB:
                    c, half = divmod(nmb, 2)
                    if half == 0:
                        xstage[c] = xs_pool.tile([P, 2, K], FP8, tag="xs",
                                                 name="xsg")
                    gmb = nmb - 4 if nmb <= 7 else nmb - 3
                    nc.gpsimd.tensor_copy(
                        out=xstage[c][:, half, 0:2],
                        in_=obs[gmb][:, N_SH - 2:N_SH])
                    nc.gpsimd.dma_start(
                        out=xstage[c][:, half, :], in_=x3d[:, c, half, :])
                    sign_x_half(c, half)
                    transpose_x_half(c, half)
                last = (mb == MB - 1)
                ob = ob_pool.tile([P, N_SH], I16, tag="ob")
                obs[mb] = ob
                # the last m-block tapers its final chunks so the exposed
                # end-of-program evict+DMA chain is short
                widths = [(0, 512), (1, 512), (2, 512), (3, 256), (3, 256)] \
                    if last else [(s, 512) for s in range(NB)]
                off_in_s = 0
                prev_s = 0
                for s, wd in widths:
                    if s != prev_s:
                        off_in_s = 0
                        prev_s = s
                    po = psum_pool.tile([P, 512], F32, tag="po", name="po")
                    for j in range(JB):
                        mm(po, mb, j, s, start=(j == 0), stop=(j == JB - 1),
                           off=off_in_s, wd=wd)
                    gate_po[mb] = po
                    nsl = slice(512 * s + off_in_s, 512 * s + off_in_s + wd)
                    nc.scalar.copy(out=ob[:, nsl], in_=po[:, 0:wd])
                    if last:
                        # overlap the two final stores on different queues
                        eng = nc.scalar if wd == 448 else nc.sync
                        eng.dma_start(
                            out=out[mb * P:(mb + 1) * P, nsl], in_=ob[:, nsl])
                    off_in_s += wd
                if not last:
                    nc.scalar.dma_start(
                        out=out[mb * P:(mb + 1) * P, :], in_=ob[:])

    nc.compile()
    return nc


def get_nc():
    global _NC_CACHE
    if _NC_CACHE is None:
        _NC_CACHE = build_nc()
    return _NC_CACHE


def kernel(x: np.ndarray, w: np.ndarray) -> np.ndarray:
    x = np.asarray(x, dtype=np.float32)
    w = np.asarray(w, dtype=np.float32)
    assert x.shape == (M_FULL, K) and w.shape == (K, N_FULL)

    nc = get_nc()
    in_maps = []
    for c in range(N_CORES):
        mi, ni = divmod(c, RN)
        # SwInterleave reads stationary columns in reverse order: pre-reverse
        # x rows within each 128-row block so output rows land in order.
        xs = x[mi * M_SH:(mi + 1) * M_SH, :]
        xs = xs.reshape(MB, P, K)[:, ::-1, :].reshape(M_SH, K)
        in_maps.append({
            "x": np.ascontiguousarray(xs),
            "w": np.ascontiguousarray(w[:, ni * N_SH:(ni + 1) * N_SH]),
        })
    res = run_bass_kernel_spmd(nc, in_maps, list(range(N_CORES)))

    out = np.empty((M_FULL, N_FULL), dtype=np.float32)
    for c in range(N_CORES):
        mi, ni = divmod(c, RN)
        out[mi * M_SH:(mi + 1) * M_SH, ni * N_SH:(ni + 1) * N_SH] = \
            res.results[c]["out"].astype(np.float32)
    return out
